# revision 9
# baseline (speedup 1.0000x reference)
"""Multi-head causal attention (B=2, T=2048, D=1024, H=16) on 8 trn2 cores.

Sharding: core c = (batch b, head-group g) with b = c//4, g = c%4.
Each core computes Q/K/V projections for its 4 heads (256 features),
causal attention, and its partial output projection; the host sums the
4 per-batch partials (the w_o all-reduce) and stacks batches.

v3: diagonal trimming + DMA/emission resequencing on top of v2.
  - Diagonal chunks only compute the live q-range (512-128i wide):
    S/exp/PV/den all narrowed; the causal mask shrinks to the single
    straddling 128-q block per head.
  - First weight/x DMAs are kd-split so the first projection starts
    as soon as half of wq/xq0 has landed; wo rides late in the queue.
  - All projection / V-proj / out-proj work is scheduled as small
    per-chunk-step fillers matched to the DMA arrival cascade.
  - Final stripe's norm runs in 128-col quarters feeding the four tail
    out-projections; out DMAs go per 512-col half.
"""

import math

import ml_dtypes
import numpy as np

BF16NP = ml_dtypes.bfloat16

import concourse.bass as bass
from concourse import bacc
import concourse.mybir as mybir
import concourse.tile as tile
from concourse.bass_utils import run_bass_kernel_spmd

F32 = mybir.dt.float32
F16 = mybir.dt.float16
AF = mybir.ActivationFunctionType
ALU = mybir.AluOpType
BF16 = mybir.dt.bfloat16

B, T, D, H = 2, 2048, 1024, 16
NCORES = 8
G = 4             # head groups (tensor parallel); cores = B * G
HPG = H // G      # 4 heads per core
DH = D // H       # 64 head dim
E = D // G        # 256 features per core
EB = E // 128     # 2 e-blocks of 128 (= head pairs)
KD = D // 128     # 8 contraction chunks for projections
TN = T // 512     # 4 512-wide stripes
TC = T // 128     # 16 128-wide k/t chunks


def build_nc():
    nc = bacc.Bacc(None)
    xqT = nc.declare_dram_parameter("xqT", [D, T], BF16, isOutput=False)
    xkT = nc.declare_dram_parameter("xkT", [D, T], BF16, isOutput=False)
    xvT = nc.declare_dram_parameter("xvT", [D, T], BF16, isOutput=False)
    wqT = nc.declare_dram_parameter("wqT", [D, E], BF16, isOutput=False)
    wkT = nc.declare_dram_parameter("wkT", [D, E], BF16, isOutput=False)
    wvT = nc.declare_dram_parameter("wvT", [D, E], BF16, isOutput=False)
    woT = nc.declare_dram_parameter("woT", [E, D], BF16, isOutput=False)
    outp = nc.declare_dram_parameter("outp", [T, D], F16, isOutput=True)

    with tile.TileContext(nc) as tc:
        with (
            tc.tile_pool(name="persist", bufs=1) as persist,
            tc.tile_pool(name="xs", bufs=11) as xs,
            tc.tile_pool(name="pt", bufs=12) as ptp,
            tc.tile_pool(name="rs", bufs=3) as rsp,
            tc.tile_pool(name="outs", bufs=4) as outsp,
            tc.tile_pool(name="psa", bufs=1, space="PSUM") as psa,
            tc.tile_pool(name="psb", bufs=1, space="PSUM") as psb,
            tc.tile_pool(name="po", bufs=1, space="PSUM") as pop,
            tc.tile_pool(name="pd", bufs=1, space="PSUM") as pdp,
            tc.tile_pool(name="pj", bufs=2, space="PSUM") as pjp,
        ):
            wq_sb = persist.tile([128, KD, E], BF16, tag="wq")
            wk_sb = persist.tile([128, KD, E], BF16, tag="wk")
            wv_sb = persist.tile([128, KD, E], BF16, tag="wv")
            wo_sb = persist.tile([128, EB, D], BF16, tag="wo")
            ones64 = persist.tile([128, 64], BF16, tag="ones")

            QTn = [[persist.tile([128, 512], BF16, tag=f"qt{eb}{n}", name=f"qt{eb}{n}")
                    for n in range(TN)] for eb in range(EB)]
            KTn = [[persist.tile([128, 512], BF16, tag=f"kt{eb}{n}", name=f"kt{eb}{n}")
                    for n in range(TN)] for eb in range(EB)]
            ONn = [[persist.tile([128, 512], BF16, tag=f"on{eb}{n}", name=f"on{eb}{n}")
                    for n in range(TN)] for eb in range(EB)]
            Vpn = [persist.tile([128, 4, HPG, DH], BF16, tag=f"vp{n}", name=f"vp{n}")
                   for n in range(TN)]

            nc.gpsimd.memset(ones64[:], 1.0)
            # dummy exp: pulls the ACT table load off the first real
            # exp call's critical path
            dum = persist.tile([1, 2], BF16, tag="dum")
            nc.scalar.activation(dum[:, :], ones64[0:1, 0:2], AF.Exp)
            # HAM warm-up: keep the PE busy through the input-DMA window
            # so the clock gate ramps to 8/8 before the first projection
            wup = pjp.tile([128, 64], F32, tag="pj")
            for _ in range(100):
                nc.tensor.matmul(wup[0:64, :], ones64[:, :], ones64[:, :],
                                 start=True, stop=True)

            # ---- DMA emission (order == queue order == arrival order) ----
            def dma_w_half(dst, src, c0, c1):
                nc.sync.dma_start(
                    dst[:, c0:c1, :],
                    src[:, :].rearrange("(c p) e -> p c e", p=128)[:, c0:c1, :],
                )

            def emit_x_dma(xdram, n, tile_=None, c0=0, c1=KD):
                t = tile_ if tile_ is not None else xs.tile([128, KD, 512], BF16, tag="x")
                nc.sync.dma_start(
                    t[:, c0:c1, :],
                    xdram[:, :].rearrange("(c p) t -> p c t", p=128)[
                        :, c0:c1, 512 * n : 512 * n + 512]
                )
                return t

            # first q/k loads are kd-split so projections start early
            dma_w_half(wq_sb, wqT, 0, 4)
            xq0 = xs.tile([128, KD, 512], BF16, tag="x")
            emit_x_dma(xqT, 0, xq0, 0, 4)
            dma_w_half(wq_sb, wqT, 4, 8)
            emit_x_dma(xqT, 0, xq0, 4, 8)
            dma_w_half(wk_sb, wkT, 0, 4)
            xk0 = xs.tile([128, KD, 512], BF16, tag="x")
            emit_x_dma(xkT, 0, xk0, 0, 4)
            dma_w_half(wk_sb, wkT, 4, 8)
            emit_x_dma(xkT, 0, xk0, 4, 8)
            nc.sync.dma_start(wv_sb[:], wvT[:, :].rearrange("(c p) e -> p c e", p=128))
            xv0 = emit_x_dma(xvT, 0)
            xst = {0: (xq0, xk0, xv0)}
            xst[1] = (emit_x_dma(xqT, 1), emit_x_dma(xkT, 1), emit_x_dma(xvT, 1))
            xq2 = emit_x_dma(xqT, 2)
            xk2 = emit_x_dma(xkT, 2)
            nc.sync.dma_start(wo_sb[:], woT[:, :].rearrange("(c p) d -> p c d", p=128))
            xst[2] = (xq2, xk2, emit_x_dma(xvT, 2))
            xst[3] = (emit_x_dma(xqT, 3), emit_x_dma(xkT, 3), emit_x_dma(xvT, 3))

            # ---- building blocks ----
            def emit_qk_proj_half(eb, n, xt, wsb, dest, half, accs):
                """Half a projection: kd 0-3 (half 0) or 4-7 (half 1 + copy)."""
                with tc.high_priority(offset=25000):
                    if half == 0:
                        acc = pjp.tile([128, 512], F32, tag="pj")
                        accs[(eb, n, id(dest))] = acc
                    else:
                        acc = accs.pop((eb, n, id(dest)))
                    for kd in range(4 * half, 4 * half + 4):
                        nc.tensor.matmul(
                            acc[:],
                            wsb[:, kd, 128 * eb : 128 * eb + 128],
                            xt[:, kd, :],
                            start=(kd == 0),
                            stop=(kd == KD - 1),
                        )
                    if half == 1:
                        nc.vector.tensor_copy(dest[eb][n][:, :], acc[:])

            qk_accs = {}

            def qk(eb, n, xt, wsb, dest, half):
                return lambda: emit_qk_proj_half(eb, n, xt, wsb, dest, half, qk_accs)

            def emit_v_proj_sub(n, sub, xt):
                # feeds the PV chain (priority 10000): keep above ops
                with tc.high_priority(offset=15000):
                    acc = pjp.tile([128, E], F32, tag="pj")
                    for kd in range(KD):
                        nc.tensor.matmul(
                            acc[:],
                            xt[:, kd, 128 * sub : 128 * sub + 128],
                            wv_sb[:, kd, :],
                            start=(kd == 0),
                            stop=(kd == KD - 1),
                        )
                    nc.vector.tensor_copy(
                        Vpn[n][:, sub, :, :],
                        acc[:].rearrange("p (h d) -> p h d", h=HPG),
                    )

            obts = {}

            def emit_outproj_half(tn, dn):
                if dn == 0:
                    obt = outsp.tile([128, 1024], F16, tag="ob")
                    obts[tn] = obt
                else:
                    obt = obts.pop(tn)
                acc = pjp.tile([128, 512], F32, tag="pj")
                for eb in range(EB):
                    nc.tensor.matmul(
                        acc[:],
                        ONn[eb][tn // 4][:, 128 * (tn % 4) : 128 * (tn % 4) + 128],
                        wo_sb[:, eb, 512 * dn : 512 * dn + 512],
                        start=(eb == 0),
                        stop=(eb == EB - 1),
                    )
                if tn >= 12 and dn == 1:
                    # after the last exps the scalar engine is free:
                    # parallelize the tail casts across engines
                    nc.scalar.copy(obt[:, 512 * dn : 512 * dn + 512], acc[:])
                else:
                    nc.vector.tensor_copy(obt[:, 512 * dn : 512 * dn + 512], acc[:])
                nc.sync.dma_start(
                    outp[128 * tn : 128 * tn + 128, 512 * dn : 512 * dn + 512],
                    obt[:, 512 * dn : 512 * dn + 512],
                )

            def emit_attn_jg(eb, jg, sched):
                """Attention for head pair eb, query stripe jg (512 q's).

                Chunk kc covers k in [128kc, 128kc+128).  Diagonal chunks
                (kc >= 4jg, i = kc-4jg) only touch the live query range
                [128i, 512); the causal mask applies to the single 128-q
                block straddling the diagonal.  sched maps step index ->
                list of filler closures emitted after that step.
                """
                nkc = 4 * jg + 4
                d0 = 4 * jg
                pO = pop.tile([128, 512], F32, tag="po")
                pD = pdp.tile([128, 512], F32, tag="pd")

                def qlo_of(kc):
                    return 128 * (kc - d0) if kc >= d0 else 0

                def emit_S(kc):
                    qlo = qlo_of(kc)
                    pool = psa if kc % 2 == 0 else psb
                    pS = pool.tile([128, 1024], F32, tag="ps", name="pS")
                    with tc.high_priority(offset=50000):
                        for h in range(2):
                            r0 = 64 * h
                            nc.tensor.matmul(
                                pS[:, 512 * h + qlo : 512 * h + 512],
                                KTn[eb][kc // 4][r0 : r0 + 64,
                                                 128 * (kc % 4) : 128 * (kc % 4) + 128],
                                QTn[eb][jg][r0 : r0 + 64, qlo:512],
                                start=True,
                                stop=True,
                            )
                    return pS

                pS_next = emit_S(0)
                for kc in range(nkc):
                    qlo = qlo_of(kc)
                    pS = pS_next
                    ptb = ptp.tile([128, 1024], BF16, tag="pt")
                    ptb3 = ptb[:, :].rearrange("p (h q) -> p h q", h=2)
                    pS3 = pS[:, :].rearrange("p (h q) -> p h q", h=2)
                    nc.scalar.activation(
                        ptb3[:, :, qlo:512], pS3[:, :, qlo:512], AF.Exp
                    )
                    if kc >= d0:
                        # causal mask on the 128-q block straddling the
                        # diagonal (one call covers both heads)
                        nc.gpsimd.affine_select(
                            out=ptb3[:, :, qlo : qlo + 128],
                            in_=ptb3[:, :, qlo : qlo + 128],
                            pattern=[[0, 2], [1, 128]],
                            compare_op=ALU.is_ge,
                            fill=0.0,
                            base=0,
                            channel_multiplier=-1,
                        )
                    if kc + 1 < nkc:
                        pS_next = emit_S(kc + 1)

                    def pv_mm(h):
                        nc.tensor.matmul(
                            pO[64 * h : 64 * h + 64, qlo:512],
                            Vpn[kc // 4][:, kc % 4, 2 * eb + h, :],
                            ptb[:, 512 * h + qlo : 512 * h + 512],
                            start=(kc == 0),
                            stop=(kc == nkc - 1),
                            skip_group_check=True,
                        )
                    def den_mm(h):
                        nc.tensor.matmul(
                            pD[64 * h : 64 * h + 64, qlo:512],
                            ones64[:, :],
                            ptb[:, 512 * h + qlo : 512 * h + 512],
                            start=(kc == 0),
                            stop=(kc == nkc - 1),
                            skip_group_check=True,
                        )
                    # PV/den recycle ptb slots the exp chain needs (pool
                    # slots are acquired in order): keep them above filler
                    # priority or the exp chain throttles at stripe ends
                    with tc.high_priority(offset=10000):
                        pv_mm(0); den_mm(1)
                        pv_mm(1); den_mm(0)
                    for fn in sched.get(kc, ()):
                        fn()
                # normalization: fast recip of the broadcast denominator,
                # then scale.  The final stripe is split into 128-col
                # quarters so each tail out-projection starts as soon as
                # its quarter lands.
                r = rsp.tile([128, 512], F32, tag="rs")
                quarters = 4 if (eb, jg) == (1, 3) else 1
                w = 512 // quarters
                # norm frees pO/pD for the next stripe's PV chain: high
                # priority so the single-buffered psum doesn't bottleneck
                with tc.high_priority(offset=10000):
                    for hv in range(quarters):
                        sl = slice(w * hv, w * hv + w)
                        nc.vector.reciprocal_approx_fast(out=r[:, sl], in_=pD[:, sl])
                        nc.vector.tensor_tensor(
                            out=ONn[eb][jg][:, sl], in0=pO[:, sl], in1=r[:, sl],
                            op=ALU.mult,
                        )

            # ---- emission schedule ----
            # stripe-0 q/k projections first (kd-split DMAs feed them)
            emit_qk_proj_half(0, 0, xq0, wq_sb, QTn, 0, qk_accs)
            emit_qk_proj_half(0, 0, xq0, wq_sb, QTn, 1, qk_accs)
            emit_qk_proj_half(0, 0, xk0, wk_sb, KTn, 0, qk_accs)
            emit_qk_proj_half(0, 0, xk0, wk_sb, KTn, 1, qk_accs)
            emit_qk_proj_half(1, 0, xq0, wq_sb, QTn, 0, qk_accs)
            emit_qk_proj_half(1, 0, xq0, wq_sb, QTn, 1, qk_accs)
            emit_qk_proj_half(1, 0, xk0, wk_sb, KTn, 0, qk_accs)
            emit_qk_proj_half(1, 0, xk0, wk_sb, KTn, 1, qk_accs)

            vp = lambda n, s: (lambda: emit_v_proj_sub(n, s, xst[n][2]))
            op = lambda t, dn: (lambda: emit_outproj_half(t, dn))

            def qh(eb, n, half):
                return qk(eb, n, xst[n][0], wq_sb, QTn, half)

            def kh(eb, n, half):
                return qk(eb, n, xst[n][1], wk_sb, KTn, half)

            # stripe-0 V projections must precede (0,0)'s PV consumers
            for s in range(4):
                emit_v_proj_sub(0, s, xv0)

            # vp(n, s) is consumed by PV at chunk kc = 4n+s, so it rides
            # at step s of the first stripe whose chunk range reaches 4n.
            emit_attn_jg(0, 0, {0: [qh(0, 1, 0)], 1: [qh(0, 1, 1)],
                                2: [kh(0, 1, 0)], 3: [kh(0, 1, 1)]})
            emit_attn_jg(1, 0, {0: [qh(1, 1, 0)], 1: [qh(1, 1, 1)],
                                2: [kh(1, 1, 0)], 3: [kh(1, 1, 1)]})
            emit_attn_jg(0, 1, {0: [vp(1, 0)], 1: [vp(1, 1)],
                                2: [vp(1, 2)], 3: [vp(1, 3)],
                                4: [qh(0, 2, 0)], 5: [qh(0, 2, 1)],
                                6: [kh(0, 2, 0)], 7: [kh(0, 2, 1)]})
            emit_attn_jg(1, 1, {0: [qh(1, 2, 0)], 1: [qh(1, 2, 1)],
                                2: [kh(1, 2, 0)], 3: [kh(1, 2, 1)],
                                4: [op(0, 0)], 5: [op(0, 1)],
                                6: [op(1, 0)], 7: [op(1, 1)]})
            emit_attn_jg(0, 2, {0: [vp(2, 0)], 1: [vp(2, 1)],
                                2: [vp(2, 2)], 3: [vp(2, 3)],
                                4: [qh(0, 3, 0)], 5: [qh(0, 3, 1)],
                                6: [kh(0, 3, 0)], 7: [kh(0, 3, 1)],
                                8: [op(2, 0)], 9: [op(2, 1)],
                                10: [op(3, 0)], 11: [op(3, 1)]})
            emit_attn_jg(1, 2, {0: [qh(1, 3, 0)], 1: [qh(1, 3, 1)],
                                2: [kh(1, 3, 0)], 3: [kh(1, 3, 1)],
                                4: [op(4, 0)], 5: [op(4, 1)],
                                6: [op(5, 0)], 7: [op(5, 1)],
                                8: [op(6, 0)], 9: [op(6, 1)],
                                10: [op(7, 0)], 11: [op(7, 1)]})
            emit_attn_jg(0, 3, {0: [vp(3, 0)], 1: [vp(3, 1)],
                                2: [vp(3, 2)], 3: [vp(3, 3)],
                                4: [op(8, 0)], 5: [op(8, 1)],
                                6: [op(9, 0)], 7: [op(9, 1)],
                                8: [op(10, 0)], 9: [op(10, 1)],
                                10: [op(11, 0)], 11: [op(11, 1)]})
            emit_attn_jg(1, 3, {})
            # tail out-projections: emitted after (1,3) so their reads
            # depend on the per-quarter norm writes of ONn[1][3]
            for t in range(12, 16):
                emit_outproj_half(t, 0)
                emit_outproj_half(t, 1)
    nc.compile()
    return nc


_CACHE = {}
LAST_RESULTS = None


def get_nc():
    if "nc" not in _CACHE:
        _CACHE["nc"] = build_nc()
    return _CACHE["nc"]


def make_in_maps(q, k, v, wq, wk, wv, wo):
    q, k, v, wq, wk, wv, wo = (
        np.asarray(a, dtype=np.float32) for a in (q, k, v, wq, wk, wv, wo)
    )
    scale = 1.0 / math.sqrt(DH)
    xT = [
        (
            np.ascontiguousarray(q[b].T).astype(BF16NP),
            np.ascontiguousarray(k[b].T).astype(BF16NP),
            np.ascontiguousarray(v[b].T).astype(BF16NP),
        )
        for b in range(B)
    ]
    in_maps = []
    for c in range(NCORES):
        b, g = divmod(c, G)
        gs = slice(E * g, E * (g + 1))
        in_maps.append(
            {
                "xqT": xT[b][0],
                "xkT": xT[b][1],
                "xvT": xT[b][2],
                "wqT": np.ascontiguousarray((wq[gs] * scale).T).astype(BF16NP),
                "wkT": np.ascontiguousarray(wk[gs].T).astype(BF16NP),
                "wvT": np.ascontiguousarray(wv[gs].T).astype(BF16NP),
                "woT": np.ascontiguousarray(wo[:, gs].T).astype(BF16NP),
            }
        )
    return in_maps


def kernel(q, k, v, wq, wk, wv, wo):
    global LAST_RESULTS
    nc = get_nc()
    in_maps = make_in_maps(q, k, v, wq, wk, wv, wo)
    res = run_bass_kernel_spmd(nc, in_maps, core_ids=list(range(NCORES)))
    LAST_RESULTS = res
    out = np.zeros((B, T, D), dtype=np.float32)
    for c in range(NCORES):
        out[c // G] += np.asarray(res.results[c]["outp"], dtype=np.float32)
    return out


# revision 12
# speedup vs baseline: 1.0486x; 1.0486x over previous
"""Multi-head causal attention (B=2, T=2048, D=1024, H=16) on 8 trn2 cores.

Sharding: core c = (batch b, head-group g) with b = c//4, g = c%4.
Each core computes Q/K/V projections for its 4 heads (256 features),
causal attention, and its partial output projection; the host sums the
4 per-batch partials (the w_o all-reduce) and stacks batches.

v3: diagonal trimming + DMA/emission resequencing on top of v2.
  - Diagonal chunks only compute the live q-range (512-128i wide):
    S/exp/PV/den all narrowed; the causal mask shrinks to the single
    straddling 128-q block per head.
  - First weight/x DMAs are kd-split so the first projection starts
    as soon as half of wq/xq0 has landed; wo rides late in the queue.
  - All projection / V-proj / out-proj work is scheduled as small
    per-chunk-step fillers matched to the DMA arrival cascade.
  - Final stripe's norm runs in 128-col quarters feeding the four tail
    out-projections; out DMAs go per 512-col half.
"""

import math

import ml_dtypes
import numpy as np

BF16NP = ml_dtypes.bfloat16

import concourse.bass as bass
from concourse import bacc
import concourse.mybir as mybir
import concourse.tile as tile
from concourse.bass_utils import run_bass_kernel_spmd

F32 = mybir.dt.float32
F16 = mybir.dt.float16
AF = mybir.ActivationFunctionType
ALU = mybir.AluOpType
BF16 = mybir.dt.bfloat16

B, T, D, H = 2, 2048, 1024, 16
NCORES = 8
G = 4             # head groups (tensor parallel); cores = B * G
HPG = H // G      # 4 heads per core
DH = D // H       # 64 head dim
E = D // G        # 256 features per core
EB = E // 128     # 2 e-blocks of 128 (= head pairs)
KD = D // 128     # 8 contraction chunks for projections
TN = T // 512     # 4 512-wide stripes
TC = T // 128     # 16 128-wide k/t chunks


def build_nc():
    nc = bacc.Bacc(None)
    xqT = nc.declare_dram_parameter("xqT", [D, T], BF16, isOutput=False)
    xkT = nc.declare_dram_parameter("xkT", [D, T], BF16, isOutput=False)
    xvT = nc.declare_dram_parameter("xvT", [D, T], BF16, isOutput=False)
    wqT = nc.declare_dram_parameter("wqT", [D, E], BF16, isOutput=False)
    wkT = nc.declare_dram_parameter("wkT", [D, E], BF16, isOutput=False)
    wvT = nc.declare_dram_parameter("wvT", [D, E], BF16, isOutput=False)
    woT = nc.declare_dram_parameter("woT", [E, D], BF16, isOutput=False)
    outp = nc.declare_dram_parameter("outp", [T, D], F16, isOutput=True)

    with tile.TileContext(nc) as tc:
        with (
            tc.tile_pool(name="persist", bufs=1) as persist,
            tc.tile_pool(name="xs", bufs=11) as xs,
            tc.tile_pool(name="pt", bufs=20) as ptp,
            tc.tile_pool(name="rs", bufs=3) as rsp,
            tc.tile_pool(name="outs", bufs=4) as outsp,
            tc.tile_pool(name="psa", bufs=1, space="PSUM") as psa,
            tc.tile_pool(name="psb", bufs=1, space="PSUM") as psb,
            tc.tile_pool(name="po", bufs=1, space="PSUM") as pop,
            tc.tile_pool(name="pd", bufs=1, space="PSUM") as pdp,
            tc.tile_pool(name="pj", bufs=2, space="PSUM") as pjp,
        ):
            wq_sb = persist.tile([128, KD, E], BF16, tag="wq")
            wk_sb = persist.tile([128, KD, E], BF16, tag="wk")
            wv_sb = persist.tile([128, KD, E], BF16, tag="wv")
            wo_sb = persist.tile([128, EB, D], BF16, tag="wo")
            ones64 = persist.tile([128, 64], BF16, tag="ones")

            QTn = [[persist.tile([128, 512], BF16, tag=f"qt{eb}{n}", name=f"qt{eb}{n}")
                    for n in range(TN)] for eb in range(EB)]
            KTn = [[persist.tile([128, 512], BF16, tag=f"kt{eb}{n}", name=f"kt{eb}{n}")
                    for n in range(TN)] for eb in range(EB)]
            ONn = [[persist.tile([128, 512], BF16, tag=f"on{eb}{n}", name=f"on{eb}{n}")
                    for n in range(TN)] for eb in range(EB)]
            Vpn = [persist.tile([128, 4, HPG, DH], BF16, tag=f"vp{n}", name=f"vp{n}")
                   for n in range(TN)]

            nc.gpsimd.memset(ones64[:], 1.0)
            # dummy exp: pulls the ACT table load off the first real
            # exp call's critical path
            dum = persist.tile([1, 2], BF16, tag="dum")
            nc.scalar.activation(dum[:, :], ones64[0:1, 0:2], AF.Exp)
            # HAM warm-up: keep the PE busy through the input-DMA window
            # so the clock gate ramps to 8/8 before the first projection
            # warmup rides in the S psum pool, NOT pjp: pjp's 2 FIFO slots
            # pace the whole projection pipeline and must not be held here
            wup = psa.tile([128, 1024], F32, tag="ps")
            for _ in range(100):
                nc.tensor.matmul(wup[0:64, 0:64], ones64[:, :], ones64[:, :],
                                 start=True, stop=True)

            # ---- DMA emission (order == queue order == arrival order) ----
            def dma_w_half(dst, src, c0, c1):
                nc.sync.dma_start(
                    dst[:, c0:c1, :],
                    src[:, :].rearrange("(c p) e -> p c e", p=128)[:, c0:c1, :],
                )

            def emit_x_dma(xdram, n, tile_=None, c0=0, c1=KD):
                t = tile_ if tile_ is not None else xs.tile([128, KD, 512], BF16, tag="x")
                nc.sync.dma_start(
                    t[:, c0:c1, :],
                    xdram[:, :].rearrange("(c p) t -> p c t", p=128)[
                        :, c0:c1, 512 * n : 512 * n + 512]
                )
                return t

            # first q/k loads are kd-split so projections start early
            dma_w_half(wq_sb, wqT, 0, 4)
            xq0 = xs.tile([128, KD, 512], BF16, tag="x")
            emit_x_dma(xqT, 0, xq0, 0, 4)
            dma_w_half(wq_sb, wqT, 4, 8)
            emit_x_dma(xqT, 0, xq0, 4, 8)
            dma_w_half(wk_sb, wkT, 0, 4)
            xk0 = xs.tile([128, KD, 512], BF16, tag="x")
            emit_x_dma(xkT, 0, xk0, 0, 4)
            dma_w_half(wk_sb, wkT, 4, 8)
            emit_x_dma(xkT, 0, xk0, 4, 8)
            nc.sync.dma_start(wv_sb[:], wvT[:, :].rearrange("(c p) e -> p c e", p=128))
            xv0 = emit_x_dma(xvT, 0)
            xst = {0: (xq0, xk0, xv0)}
            xst[1] = (emit_x_dma(xqT, 1), emit_x_dma(xkT, 1), emit_x_dma(xvT, 1))
            xq2 = emit_x_dma(xqT, 2)
            xk2 = emit_x_dma(xkT, 2)
            nc.sync.dma_start(wo_sb[:], woT[:, :].rearrange("(c p) d -> p c d", p=128))
            xst[2] = (xq2, xk2, emit_x_dma(xvT, 2))
            xst[3] = (emit_x_dma(xqT, 3), emit_x_dma(xkT, 3), emit_x_dma(xvT, 3))

            # ---- building blocks ----
            def emit_qk_proj_half(eb, n, xt, wsb, dest, half, accs):
                """Half a projection: kd 0-3 (half 0) or 4-7 (half 1 + copy)."""
                with tc.high_priority(offset=25000):
                    if half == 0:
                        acc = pjp.tile([128, 512], F32, tag="pj")
                        accs[(eb, n, id(dest))] = acc
                    else:
                        acc = accs.pop((eb, n, id(dest)))
                    for kd in range(4 * half, 4 * half + 4):
                        nc.tensor.matmul(
                            acc[:],
                            wsb[:, kd, 128 * eb : 128 * eb + 128],
                            xt[:, kd, :],
                            start=(kd == 0),
                            stop=(kd == KD - 1),
                        )
                    if half == 1:
                        nc.vector.tensor_copy(dest[eb][n][:, :], acc[:])

            qk_accs = {}

            def qk(eb, n, xt, wsb, dest, half):
                return lambda: emit_qk_proj_half(eb, n, xt, wsb, dest, half, qk_accs)

            def emit_v_proj_sub(n, sub, xt):
                acc = pjp.tile([128, E], F32, tag="pj")
                for kd in range(KD):
                    nc.tensor.matmul(
                        acc[:],
                        xt[:, kd, 128 * sub : 128 * sub + 128],
                        wv_sb[:, kd, :],
                        start=(kd == 0),
                        stop=(kd == KD - 1),
                    )
                nc.vector.tensor_copy(
                    Vpn[n][:, sub, :, :],
                    acc[:].rearrange("p (h d) -> p h d", h=HPG),
                )

            obts = {}

            def emit_outproj_half(tn, dn):
                if dn == 0:
                    obt = outsp.tile([128, 1024], F16, tag="ob")
                    obts[tn] = obt
                else:
                    obt = obts.pop(tn)
                acc = pjp.tile([128, 512], F32, tag="pj")
                for eb in range(EB):
                    nc.tensor.matmul(
                        acc[:],
                        ONn[eb][tn // 4][:, 128 * (tn % 4) : 128 * (tn % 4) + 128],
                        wo_sb[:, eb, 512 * dn : 512 * dn + 512],
                        start=(eb == 0),
                        stop=(eb == EB - 1),
                    )
                if tn >= 12 and dn == 1:
                    # after the last exps the scalar engine is free:
                    # parallelize the tail casts across engines
                    nc.scalar.copy(obt[:, 512 * dn : 512 * dn + 512], acc[:])
                else:
                    nc.vector.tensor_copy(obt[:, 512 * dn : 512 * dn + 512], acc[:])
                nc.sync.dma_start(
                    outp[128 * tn : 128 * tn + 128, 512 * dn : 512 * dn + 512],
                    obt[:, 512 * dn : 512 * dn + 512],
                )

            def emit_attn_jg(eb, jg, sched):
                """Attention for head pair eb, query stripe jg (512 q's).

                Chunk kc covers k in [128kc, 128kc+128).  Diagonal chunks
                (kc >= 4jg, i = kc-4jg) only touch the live query range
                [128i, 512); the causal mask applies to the single 128-q
                block straddling the diagonal.  sched maps step index ->
                list of filler closures emitted after that step.
                """
                nkc = 4 * jg + 4
                d0 = 4 * jg
                pO = pop.tile([128, 512], F32, tag="po")
                pD = pdp.tile([128, 512], F32, tag="pd")

                def qlo_of(kc):
                    return 128 * (kc - d0) if kc >= d0 else 0

                def emit_S(kc):
                    qlo = qlo_of(kc)
                    pool = psa if kc % 2 == 0 else psb
                    pS = pool.tile([128, 1024], F32, tag="ps", name="pS")
                    with tc.high_priority(offset=50000):
                        for h in range(2):
                            r0 = 64 * h
                            nc.tensor.matmul(
                                pS[:, 512 * h + qlo : 512 * h + 512],
                                KTn[eb][kc // 4][r0 : r0 + 64,
                                                 128 * (kc % 4) : 128 * (kc % 4) + 128],
                                QTn[eb][jg][r0 : r0 + 64, qlo:512],
                                start=True,
                                stop=True,
                            )
                    return pS

                pS_next = emit_S(0)
                for kc in range(nkc):
                    qlo = qlo_of(kc)
                    pS = pS_next
                    ptb = ptp.tile([128, 1024], BF16, tag="pt")
                    ptb3 = ptb[:, :].rearrange("p (h q) -> p h q", h=2)
                    pS3 = pS[:, :].rearrange("p (h q) -> p h q", h=2)
                    nc.scalar.activation(
                        ptb3[:, :, qlo:512], pS3[:, :, qlo:512], AF.Exp
                    )
                    if kc >= d0:
                        # causal mask on the 128-q block straddling the
                        # diagonal (one call covers both heads)
                        nc.gpsimd.affine_select(
                            out=ptb3[:, :, qlo : qlo + 128],
                            in_=ptb3[:, :, qlo : qlo + 128],
                            pattern=[[0, 2], [1, 128]],
                            compare_op=ALU.is_ge,
                            fill=0.0,
                            base=0,
                            channel_multiplier=-1,
                        )
                    if kc + 1 < nkc:
                        pS_next = emit_S(kc + 1)

                    def pv_mm(h):
                        nc.tensor.matmul(
                            pO[64 * h : 64 * h + 64, qlo:512],
                            Vpn[kc // 4][:, kc % 4, 2 * eb + h, :],
                            ptb[:, 512 * h + qlo : 512 * h + 512],
                            start=(kc == 0),
                            stop=(kc == nkc - 1),
                            skip_group_check=True,
                        )
                    def den_mm(h):
                        nc.tensor.matmul(
                            pD[64 * h : 64 * h + 64, qlo:512],
                            ones64[:, :],
                            ptb[:, 512 * h + qlo : 512 * h + 512],
                            start=(kc == 0),
                            stop=(kc == nkc - 1),
                            skip_group_check=True,
                        )
                    pv_mm(0); den_mm(1)
                    pv_mm(1); den_mm(0)
                    for fn in sched.get(kc, ()):
                        fn()
                # normalization: fast recip of the broadcast denominator,
                # then scale.  The final stripe is split into 128-col
                # quarters so each tail out-projection starts as soon as
                # its quarter lands.
                r = rsp.tile([128, 512], F32, tag="rs")
                quarters = 4 if (eb, jg) == (1, 3) else 1
                w = 512 // quarters
                for hv in range(quarters):
                    sl = slice(w * hv, w * hv + w)
                    nc.vector.reciprocal_approx_fast(out=r[:, sl], in_=pD[:, sl])
                    nc.vector.tensor_tensor(
                        out=ONn[eb][jg][:, sl], in0=pO[:, sl], in1=r[:, sl],
                        op=ALU.mult,
                    )

            # ---- emission schedule ----
            # stripe-0 q/k projections first (kd-split DMAs feed them)
            emit_qk_proj_half(0, 0, xq0, wq_sb, QTn, 0, qk_accs)
            emit_qk_proj_half(0, 0, xq0, wq_sb, QTn, 1, qk_accs)
            emit_qk_proj_half(0, 0, xk0, wk_sb, KTn, 0, qk_accs)
            emit_qk_proj_half(0, 0, xk0, wk_sb, KTn, 1, qk_accs)
            emit_qk_proj_half(1, 0, xq0, wq_sb, QTn, 0, qk_accs)
            emit_qk_proj_half(1, 0, xq0, wq_sb, QTn, 1, qk_accs)
            emit_qk_proj_half(1, 0, xk0, wk_sb, KTn, 0, qk_accs)
            emit_qk_proj_half(1, 0, xk0, wk_sb, KTn, 1, qk_accs)

            vp = lambda n, s: (lambda: emit_v_proj_sub(n, s, xst[n][2]))
            op = lambda t, dn: (lambda: emit_outproj_half(t, dn))

            def qh(eb, n, half):
                return qk(eb, n, xst[n][0], wq_sb, QTn, half)

            def kh(eb, n, half):
                return qk(eb, n, xst[n][1], wk_sb, KTn, half)

            # stripe-0 V projections must precede (0,0)'s PV consumers
            for s in range(4):
                emit_v_proj_sub(0, s, xv0)

            # vp(n, s) is consumed by PV at chunk kc = 4n+s, so it rides
            # at step s of the first stripe whose chunk range reaches 4n.
            emit_attn_jg(0, 0, {0: [qh(0, 1, 0)], 1: [qh(0, 1, 1)],
                                2: [kh(0, 1, 0)], 3: [kh(0, 1, 1)]})
            emit_attn_jg(1, 0, {0: [qh(1, 1, 0)], 1: [qh(1, 1, 1)],
                                2: [kh(1, 1, 0)], 3: [kh(1, 1, 1)]})
            emit_attn_jg(0, 1, {0: [vp(1, 0)], 1: [vp(1, 1)],
                                2: [vp(1, 2)], 3: [vp(1, 3)],
                                4: [qh(0, 2, 0)], 5: [qh(0, 2, 1)],
                                6: [kh(0, 2, 0)], 7: [kh(0, 2, 1)]})
            emit_attn_jg(1, 1, {0: [qh(1, 2, 0)], 1: [qh(1, 2, 1)],
                                2: [kh(1, 2, 0)], 3: [kh(1, 2, 1)],
                                4: [op(0, 0)], 5: [op(0, 1)],
                                6: [op(1, 0)], 7: [op(1, 1)]})
            emit_attn_jg(0, 2, {0: [vp(2, 0)], 1: [vp(2, 1)],
                                2: [vp(2, 2)], 3: [vp(2, 3)],
                                4: [qh(0, 3, 0)], 5: [qh(0, 3, 1)],
                                6: [kh(0, 3, 0)], 7: [kh(0, 3, 1)],
                                8: [op(2, 0)], 9: [op(2, 1)],
                                10: [op(3, 0)], 11: [op(3, 1)]})
            emit_attn_jg(1, 2, {0: [qh(1, 3, 0)], 1: [qh(1, 3, 1)],
                                2: [kh(1, 3, 0)], 3: [kh(1, 3, 1)],
                                4: [op(4, 0)], 5: [op(4, 1)],
                                6: [op(5, 0)], 7: [op(5, 1)],
                                8: [op(6, 0)], 9: [op(6, 1)],
                                10: [op(7, 0)], 11: [op(7, 1)]})
            emit_attn_jg(0, 3, {0: [vp(3, 0)], 1: [vp(3, 1)],
                                2: [vp(3, 2)], 3: [vp(3, 3)],
                                4: [op(8, 0)], 5: [op(8, 1)],
                                6: [op(9, 0)], 7: [op(9, 1)],
                                8: [op(10, 0)], 9: [op(10, 1)],
                                10: [op(11, 0)], 11: [op(11, 1)]})
            emit_attn_jg(1, 3, {})
            # tail out-projections: emitted after (1,3) so their reads
            # depend on the per-quarter norm writes of ONn[1][3]
            for t in range(12, 16):
                emit_outproj_half(t, 0)
                emit_outproj_half(t, 1)
    nc.compile()
    return nc


_CACHE = {}
LAST_RESULTS = None


def get_nc():
    if "nc" not in _CACHE:
        _CACHE["nc"] = build_nc()
    return _CACHE["nc"]


def make_in_maps(q, k, v, wq, wk, wv, wo):
    q, k, v, wq, wk, wv, wo = (
        np.asarray(a, dtype=np.float32) for a in (q, k, v, wq, wk, wv, wo)
    )
    scale = 1.0 / math.sqrt(DH)
    xT = [
        (
            np.ascontiguousarray(q[b].T).astype(BF16NP),
            np.ascontiguousarray(k[b].T).astype(BF16NP),
            np.ascontiguousarray(v[b].T).astype(BF16NP),
        )
        for b in range(B)
    ]
    in_maps = []
    for c in range(NCORES):
        b, g = divmod(c, G)
        gs = slice(E * g, E * (g + 1))
        in_maps.append(
            {
                "xqT": xT[b][0],
                "xkT": xT[b][1],
                "xvT": xT[b][2],
                "wqT": np.ascontiguousarray((wq[gs] * scale).T).astype(BF16NP),
                "wkT": np.ascontiguousarray(wk[gs].T).astype(BF16NP),
                "wvT": np.ascontiguousarray(wv[gs].T).astype(BF16NP),
                "woT": np.ascontiguousarray(wo[:, gs].T).astype(BF16NP),
            }
        )
    return in_maps


def kernel(q, k, v, wq, wk, wv, wo):
    global LAST_RESULTS
    nc = get_nc()
    in_maps = make_in_maps(q, k, v, wq, wk, wv, wo)
    res = run_bass_kernel_spmd(nc, in_maps, core_ids=list(range(NCORES)))
    LAST_RESULTS = res
    out = np.zeros((B, T, D), dtype=np.float32)
    for c in range(NCORES):
        out[c // G] += np.asarray(res.results[c]["outp"], dtype=np.float32)
    return out
